# revision 5
# baseline (speedup 1.0000x reference)
"""Trainium2 Bass kernel for BetterPixelBilateralFilter2.

Problem: 5x5 dilated (dilation=3) bilateral filter over [B=2, C=32, 720, 1280]
with per-pixel range coefficients pc = -exp(coeffs)*softplus(scale) and
per-pixel spatial coefficients psy/psx.  Output = first 3 filtered channels.

Sharding: 8 cores = batch(2) x W-quarter(4).  Each core handles a full-height
[720, 320] slab of one batch image.

Device layout (per core), 6 chunks of 120 rows (= 4 subchunks x 30):
  - channel stage: partitions = (subchunk g, channel c) = 4x32; free = (y, x).
    All tap shifts are free-dim view offsets.  Per tap-pair: diff (DVE),
    square (ACT), mul-by-pc (DVE).
  - channel reduce: per y-row, a matmul with a shifted view of a constant
    selection matrix (lhsT[:, p] = 1 iff p == pixel_partition(g, y))
    accumulates into ONE PSUM [128, 320] tile per tap that lands directly in
    pixel layout: partition p <-> row y = 4*(p//16) + p%4, g = (p%16)//4.
    The spatial log-weights psy*dy^2 + psx*dx^2 are added into the same PSUM
    by identity-stationary matmuls streaming host-prepared psy/psx maps.
  - pixel stage: exp straight from PSUM (ACT) -> w; t3 = w * f3 neighbors
    (DVE, single mul per tap); num (3ch) and den accumulate in PSUM via
    identity-stationary matmuls (TensorE), center tap seeds the group.
    Finals: den+1 (center weight), reciprocal, 3 muls, DMA out.
  - 8 hole partitions (y>=30 slots) carry zeros and are dropped on the host.

Border handling: host pads f with 1e4; (f - 1e4)^2 * pc <= -3e4 so exp
underflows to exactly 0 -- out-of-image taps contribute nothing.
"""

import numpy as np
import ml_dtypes

BF16 = ml_dtypes.bfloat16
PADV = 1.0e4

B, C, H, W = 2, 32, 720, 1280
NCORE = 8
WQ = 320           # x-quarter width per core
CH = 120           # rows per chunk
NG = 4             # y-subchunks per chunk
NY = 30            # rows per subchunk
NCH = H // CH      # 6 chunks
FH, FW = NY + 12, WQ + 12      # f-tile window 42 x 332 (+-6 halo)
D2H, D2W = 36, 326             # max diff-window (30+6, 320+6)
PXW = WQ + 12                  # f3 x-window 332
S0 = 113                       # selection-matrix center column
MW = S0 + 128                  # master selection matrix width

# positive tap offsets (dy,dx); each also covers its negation
POS = [(0, 1), (0, 2),
       (1, -2), (1, -1), (1, 0), (1, 1), (1, 2),
       (2, -2), (2, -1), (2, 0), (2, 1), (2, 2)]


def _sp_terms(dy, dx):
    """Spatial-map channels to add for this tap pair: dy^2*psy + dx^2*psx,
    using spyx channels [psy, psx, 4*psy, 4*psx]."""
    terms = []
    if dy * dy == 1:
        terms.append(0)
    elif dy * dy == 4:
        terms.append(2)
    if dx * dx == 1:
        terms.append(1)
    elif dx * dx == 4:
        terms.append(3)
    return terms


def _pixel_perm():
    """pperm[p] = chunk-local row (30*g + y_sub) for real partitions, -1 holes."""
    pperm = np.full(128, -1, np.int64)
    for y in range(NY):
        h, r = divmod(y, 4)
        for g in range(NG):
            pperm[16 * h + 4 * g + r] = NY * g + y
    return pperm


PPERM = _pixel_perm()          # [128], -1 at 8 hole slots
REAL = PPERM >= 0


def build_nc(n_chunks=NCH):
    import concourse.bacc as bacc
    import concourse.bass as bass
    import concourse.tile as tile
    from concourse import mybir

    def bcast_mid(a, n):
        """[P, X] view -> [P, n, X] with a stride-0 middle dim."""
        return bass.AP(tensor=a.tensor, offset=a.offset,
                       ap=[a.ap[0], [0, n], a.ap[1]])

    bf = mybir.dt.bfloat16
    f32 = mybir.dt.float32
    AF = mybir.ActivationFunctionType
    OP = mybir.AluOpType

    nc = bacc.Bacc("TRN2", num_devices=NCORE, debug=False)
    fin = nc.dram_tensor("fin", [n_chunks, 128, FH, FW], bf,
                         kind="ExternalInput").ap()
    pcin = nc.dram_tensor("pcin", [n_chunks, 128, NY, WQ], bf,
                          kind="ExternalInput").ap()
    f3in = nc.dram_tensor("f3in", [n_chunks, 128, 5, 3, PXW], bf,
                          kind="ExternalInput").ap()
    spin = nc.dram_tensor("spin", [n_chunks, 128, 4, WQ], bf,
                          kind="ExternalInput").ap()
    selin = nc.dram_tensor("selin", [128, MW], bf, kind="ExternalInput").ap()
    idin = nc.dram_tensor("idin", [128, 128], bf, kind="ExternalInput").ap()
    out = nc.dram_tensor("out", [n_chunks, 128, 3, WQ], f32,
                         kind="ExternalOutput").ap()

    with tile.TileContext(nc) as tc:
        with (
            tc.tile_pool(name="consts", bufs=1) as consts,
            tc.tile_pool(name="fload", bufs=2) as fload,
            tc.tile_pool(name="pxload", bufs=1) as pxload,
            tc.tile_pool(name="dpool", bufs=1) as dpool,
            tc.tile_pool(name="prpool", bufs=3) as prpool,
            tc.tile_pool(name="wpool", bufs=3) as wpool,
            tc.tile_pool(name="opool", bufs=1) as opool,
            tc.tile_pool(name="pspool", bufs=4, space="PSUM") as pspool,
            tc.tile_pool(name="psacc", bufs=1, space="PSUM") as psacc,
        ):
            selt = consts.tile([128, MW], bf)
            nc.sync.dma_start(out=selt, in_=selin)
            identt = consts.tile([128, 128], bf)
            nc.sync.dma_start(out=identt, in_=idin)
            onest = consts.tile([128, WQ], bf)
            nc.vector.memset(onest, 1.0)

            def emit_subsq(ft, ip):
                """Window diff (DVE) + square (ACT) for tap pair ip."""
                dy, dx = POS[ip]
                y0 = -3 * dy
                x0w = min(0, -3 * dx)
                wy = NY + 3 * dy
                wx = WQ + 3 * abs(dx)
                dft = dpool.tile([128, D2H, D2W], bf, tag="dft")
                dv = dft[:, :wy, :wx]
                i0y, i0x = 6 + y0, 6 + x0w
                i1y, i1x = 6 + y0 + 3 * dy, 6 + x0w + 3 * dx
                in0 = ft[:, i0y:i0y + wy, i0x:i0x + wx]
                in1 = ft[:, i1y:i1y + wy, i1x:i1x + wx]
                # GPSIMD is useless here: its SBUF port is an exclusive
                # lock shared with DVE, so GPSIMD tensor ops stall DVE.
                nc.vector.tensor_sub(out=dv, in0=in0, in1=in1)
                nc.scalar.activation(out=dv, in_=dv, func=AF.Square)
                return dft

            def emit_prods(pct, dft, ip):
                """pc * d2 for both signs of tap pair ip (DVE)."""
                dy, dx = POS[ip]
                prods = {}
                for sgn in (1, -1):
                    if sgn > 0:
                        ry, rx = 3 * dy, max(0, 3 * dx)
                    else:
                        ry, rx = 0, max(0, -3 * dx)
                    d2v = dft[:, ry:ry + NY, rx:rx + WQ]
                    prodt = prpool.tile([128, NY, WQ], bf, tag="prod",
                                        name=f"prod_{sgn}")
                    nc.vector.tensor_mul(out=prodt, in0=pct, in1=d2v)
                    prods[sgn] = prodt
                return prods

            def emit_yloop(spt, prods, ip):
                """Channel-reduce matmuls + spatial log-weights (PE).
                Consumes prods[+1] fully before prods[-1] so the + buffer
                frees at the halfway point for the next pair's muls."""
                dy, dx = POS[ip]
                lws = {}
                for sgn in (1, -1):
                    lws[sgn] = pspool.tile([128, WQ], f32, tag="lw",
                                           name=f"lw_{sgn}")
                for sgn in (1, -1):
                    for y in range(NY):
                        sy = S0 - (16 * (y // 4) + (y % 4))
                        nc.tensor.matmul(
                            out=lws[sgn],
                            lhsT=selt[:, sy:sy + 128],
                            rhs=prods[sgn][:, y, :],
                            start=(y == 0), stop=False,
                        )
                terms = _sp_terms(dy, dx)
                for it, ch in enumerate(terms):
                    for sgn in (1, -1):
                        nc.tensor.matmul(
                            out=lws[sgn], lhsT=identt, rhs=spt[:, ch, :],
                            start=False, stop=(it == len(terms) - 1),
                        )
                return lws

            def emit_exps(lws, wfts, ip):
                for sgn in (1, -1):
                    wft = wpool.tile([128, WQ], bf, tag=f"wft{ip}{sgn}",
                                     name=f"wft_{ip}_{sgn}", bufs=1)
                    nc.scalar.activation(out=wft, in_=lws[sgn], func=AF.Exp)
                    wfts[(ip, sgn)] = wft

            def emit_batch(f3t, wfts, pnums, pdent):
                """Center seeds + t3 muls (DVE) with identity-stationary
                accumulation matmuls (PE) trailing behind."""
                for ch in range(3):
                    nc.tensor.matmul(out=pnums[ch], lhsT=identt,
                                     rhs=f3t[:, 2, ch, 6:6 + WQ],
                                     start=True, stop=False)
                nc.tensor.matmul(out=pdent, lhsT=identt, rhs=onest,
                                 start=True, stop=False)
                for ip, (dy, dx) in enumerate(POS):
                    last = ip == len(POS) - 1
                    for sgn in (1, -1):
                        ddy, ddx = sgn * dy, sgn * dx
                        wft = wfts[(ip, sgn)]
                        t3 = wpool.tile([128, 3, WQ], bf, tag="t3",
                                        name=f"t3_{sgn}")
                        nc.vector.tensor_mul(
                            out=t3,
                            in0=bcast_mid(wft[:], 3),
                            in1=f3t[:, 2 + ddy, :,
                                    6 + 3 * ddx:6 + 3 * ddx + WQ],
                        )
                        stop = last and sgn == -1
                        for ch in range(3):
                            nc.tensor.matmul(out=pnums[ch], lhsT=identt,
                                             rhs=t3[:, ch, :],
                                             start=False, stop=stop)
                        nc.tensor.matmul(out=pdent, lhsT=identt, rhs=wft,
                                         start=False, stop=stop)

            prev_finals = None
            for j in range(n_chunks):
                ft = fload.tile([128, FH, FW], bf, tag="ft")
                pct = fload.tile([128, NY, WQ], bf, tag="pct", bufs=1)
                f3t = pxload.tile([128, 5, 3, PXW], bf, tag="f3t")
                spt = pxload.tile([128, 4, WQ], bf, tag="spt")
                nc.sync.dma_start(out=ft, in_=fin[j])
                nc.sync.dma_start(out=pct, in_=pcin[j])
                nc.sync.dma_start(out=f3t, in_=f3in[j])
                nc.sync.dma_start(out=spt, in_=spin[j])

                pnums = [psacc.tile([128, WQ], f32, tag=f"pnum{ch}",
                                    name=f"pnum{ch}") for ch in range(3)]
                pdent = psacc.tile([128, WQ], f32, tag="pden")
                wfts = {}

                # software pipeline: square of pair k+1 is emitted BEFORE
                # exp of pair k so the ACT queue never serializes the next
                # pair's diff chain behind the PE y-loop round-trip.
                dft = emit_subsq(ft, 0)
                prods = emit_prods(pct, dft, 0)
                for k in range(len(POS)):
                    lws = emit_yloop(spt, prods, k)
                    if k + 1 < len(POS):
                        dft = emit_subsq(ft, k + 1)
                        prods = emit_prods(pct, dft, k + 1)
                    emit_exps(lws, wfts, k)
                    if k == 0 and prev_finals is not None:
                        prev_finals()
                        prev_finals = None
                emit_batch(f3t, wfts, pnums, pdent)

                def make_finals(j=j, pnums=pnums, pdent=pdent):
                    def finals():
                        rden = wpool.tile([128, WQ], f32, tag="rden", bufs=1)
                        nc.vector.reciprocal(out=rden, in_=pdent)
                        osb = opool.tile([128, 3, WQ], f32, tag="osb")
                        for ch in range(3):
                            nc.vector.tensor_mul(out=osb[:, ch, :],
                                                 in0=pnums[ch], in1=rden)
                        nc.sync.dma_start(out=out[j], in_=osb)
                    return finals
                prev_finals = make_finals()
            prev_finals()

    nc.compile()
    return nc


def prep_inputs(input, coeffs, n_chunks=NCH):
    """Build per-core in_maps (list of 8 dicts of numpy arrays)."""
    inp = np.asarray(input, np.float32)
    f = inp[:, :C]                      # [2,32,720,1280]
    scale = inp[:, C:]                  # [2,34,720,1280]
    k = np.exp(np.asarray(coeffs, np.float32).reshape(-1))   # [34]
    sp = np.logaddexp(0.0, scale)
    params = -(k[None, :, None, None] * sp)
    pc = params[:, :C]
    psy = params[:, C]                  # [2,720,1280]
    psx = params[:, C + 1]

    # padded f: rows/cols shifted by +6
    fp = np.full((B, C, H + 12, W + 12), PADV, np.float32)
    fp[:, :, 6:6 + H, 6:6 + W] = f
    # padded first-3-channel f for the pixel stage: shifted by +6
    f3p = np.full((B, 3, H + 12, W + 12), PADV, np.float32)
    f3p[:, :, 6:6 + H, 6:6 + W] = f[:, :3]

    # selection master matrix: sel[(g,c), v] = 1 iff v == S0 + 4g
    sel = np.zeros((128, MW), np.float32)
    for g in range(NG):
        sel[32 * g:32 * (g + 1), S0 + 4 * g] = 1.0
    ident = np.eye(128, dtype=np.float32)

    # row-gather index with holes -> clamp to row 0 and zero later
    prow = np.where(REAL, PPERM, 0)

    in_maps = []
    for b in range(B):
        for q in range(4):
            x0 = WQ * q
            fpb = fp[b, :, :, x0:x0 + FW]          # [32, 732, 332]
            s = fpb.strides
            fin = np.lib.stride_tricks.as_strided(
                fpb, shape=(n_chunks, NG, C, FH, FW),
                strides=(CH * s[1], NY * s[1], s[0], s[1], s[2]),
            ).reshape(n_chunks, 128, FH, FW)

            pcb = pc[b, :, :, x0:x0 + WQ]          # [32, 720, 320]
            s = pcb.strides
            pcin = np.lib.stride_tricks.as_strided(
                pcb, shape=(n_chunks, NG, C, NY, WQ),
                strides=(CH * s[1], NY * s[1], s[0], s[1], s[2]),
            ).reshape(n_chunks, 128, NY, WQ)

            # f3in[j, d, p, c, xx] = f3p[b, c, 120j + prow[p] + 3(d-2) + 6, x0+xx]
            j_idx = np.arange(n_chunks)[:, None, None]
            d_idx = np.arange(5)[None, :, None]
            p_idx = prow[None, None, :]
            rows = CH * j_idx + p_idx + 3 * (d_idx - 2) + 6   # [j, d, p]
            f3in = f3p[b][:, rows, x0:x0 + PXW]               # [3, j, d, p, PXW]
            # -> [j, p, d, c, x] to match SBUF tile [128, 5, 3, PXW]
            f3in = np.ascontiguousarray(f3in.transpose(1, 3, 2, 0, 4))
            f3in[:, ~REAL] = 0.0

            # spin[j, p, m, xx]: spatial coefficient maps in pixel layout,
            # channels [psy, psx, 4*psy, 4*psx]
            rows2 = CH * np.arange(n_chunks)[:, None] + prow[None, :]  # [j, p]
            spy = psy[b][rows2, x0:x0 + WQ]                   # [j, p, WQ]
            spx = psx[b][rows2, x0:x0 + WQ]
            spin = np.stack([spy, spx, 4.0 * spy, 4.0 * spx], axis=2)
            spin[:, ~REAL] = 0.0

            in_maps.append({
                "fin": fin.astype(BF16),
                "pcin": pcin.astype(BF16),
                "f3in": f3in.astype(BF16),
                "spin": np.ascontiguousarray(spin).astype(BF16),
                "selin": sel.astype(BF16),
                "idin": ident.astype(BF16),
            })
    return in_maps


def assemble_output(results, n_chunks=NCH):
    outf = np.empty((B, 3, H, W), np.float32)
    i = 0
    for b in range(B):
        for q in range(4):
            x0 = WQ * q
            o = np.asarray(results[i]["out"], np.float32)  # [j, 128, 3, WQ]
            for j in range(n_chunks):
                # fancy-index on axis 2 with slice on axis 1 -> result axes
                # are (row, c, x), matching o[j, REAL] directly
                outf[b, :, CH * j + PPERM[REAL], x0:x0 + WQ] = o[j, REAL]
            i += 1
    return outf


_NC_CACHE = {}


def kernel(input, coeffs, kernel_size=5, dilation=3, dynamic_size=3):
    assert int(kernel_size) == 5 and int(dilation) == 3
    assert int(dynamic_size) == 3
    from concourse import bass_utils

    if "nc" not in _NC_CACHE:
        _NC_CACHE["nc"] = build_nc(NCH)
    nc = _NC_CACHE["nc"]
    in_maps = prep_inputs(input, coeffs, NCH)
    res = bass_utils.run_bass_kernel_spmd(nc, in_maps,
                                          core_ids=list(range(NCORE)))
    return assemble_output(res.results, NCH)


# revision 8
# speedup vs baseline: 1.4852x; 1.4852x over previous
"""Trainium2 Bass kernel for BetterPixelBilateralFilter2.

Problem: 5x5 dilated (dilation=3) bilateral filter over [B=2, C=32, 720, 1280]
with per-pixel range coefficients pc = -exp(coeffs)*softplus(scale) and
per-pixel spatial coefficients psy/psx.  Output = first 3 filtered channels.

Sharding: 8 cores = batch(2) x W-quarter(4).  Each core handles a full-height
[720, 320] slab of one batch image.

Device layout (per core), 6 chunks of 120 rows (= 4 subchunks x 30):
  - channel stage: partitions = (subchunk g, channel c) = 4x32; free = (y, x).
    All tap shifts are free-dim view offsets.  Per tap-pair: diff (DVE),
    square (ACT), mul-by-pc (DVE).
  - channel reduce: per y-row, a matmul with a shifted view of a constant
    selection matrix (lhsT[:, p] = 1 iff p == pixel_partition(g, y))
    accumulates into ONE PSUM [128, 320] tile per tap that lands directly in
    pixel layout: partition p <-> row y = 4*(p//16) + p%4, g = (p%16)//4.
    The spatial log-weights psy*dy^2 + psx*dx^2 are added into the same PSUM
    by identity-stationary matmuls streaming host-prepared psy/psx maps.
  - pixel stage: exp straight from PSUM (ACT) -> w; t3 = w * f3 neighbors
    (DVE, single mul per tap); num (3ch) and den accumulate in PSUM via
    identity-stationary matmuls (TensorE), center tap seeds the group.
    Finals: den+1 (center weight), reciprocal, 3 muls, DMA out.
  - 8 hole partitions (y>=30 slots) carry zeros and are dropped on the host.

Border handling: host pads f with 1e4; (f - 1e4)^2 * pc <= -3e4 so exp
underflows to exactly 0 -- out-of-image taps contribute nothing.
"""

import numpy as np
import ml_dtypes

BF16 = ml_dtypes.bfloat16
PADV = 1.0e4

B, C, H, W = 2, 32, 720, 1280
NCORE = 8
WQ = 320           # x-quarter width per core
CH = 120           # rows per chunk
NG = 4             # y-subchunks per chunk
NY = 30            # rows per subchunk
NCH = H // CH      # 6 chunks
FH, FW = NY + 12, WQ + 12      # f-tile window 42 x 332 (+-6 halo)
D2H, D2W = 36, 326             # max diff-window (30+6, 320+6)
PXW = WQ + 12                  # f3 x-window 332
S0 = 113                       # selection-matrix center column
MW = S0 + 128                  # master selection matrix width

# positive tap offsets (dy,dx); each also covers its negation
POS = [(0, 1), (0, 2),
       (1, -2), (1, -1), (1, 0), (1, 1), (1, 2),
       (2, -2), (2, -1), (2, 0), (2, 1), (2, 2)]


def _sp_terms(dy, dx):
    """Spatial-map channels to add for this tap pair: dy^2*psy + dx^2*psx,
    using spyx channels [psy, psx, 4*psy, 4*psx]."""
    terms = []
    if dy * dy == 1:
        terms.append(0)
    elif dy * dy == 4:
        terms.append(2)
    if dx * dx == 1:
        terms.append(1)
    elif dx * dx == 4:
        terms.append(3)
    return terms


def _pixel_perm():
    """pperm[p] = chunk-local row (30*g + y_sub) for real partitions, -1 holes."""
    pperm = np.full(128, -1, np.int64)
    for y in range(NY):
        h, r = divmod(y, 4)
        for g in range(NG):
            pperm[16 * h + 4 * g + r] = NY * g + y
    return pperm


PPERM = _pixel_perm()          # [128], -1 at 8 hole slots
REAL = PPERM >= 0


def build_nc(n_chunks=NCH):
    import concourse.bacc as bacc
    import concourse.bass as bass
    import concourse.tile as tile
    from concourse import mybir

    def bcast_mid(a, n):
        """[P, X] view -> [P, n, X] with a stride-0 middle dim."""
        return bass.AP(tensor=a.tensor, offset=a.offset,
                       ap=[a.ap[0], [0, n], a.ap[1]])

    bf = mybir.dt.bfloat16
    f32 = mybir.dt.float32
    AF = mybir.ActivationFunctionType
    OP = mybir.AluOpType

    nc = bacc.Bacc("TRN2", num_devices=NCORE, debug=False)
    fin = nc.dram_tensor("fin", [n_chunks, 128, FH, FW], bf,
                         kind="ExternalInput").ap()
    pcin = nc.dram_tensor("pcin", [n_chunks, 128, NY, WQ], bf,
                          kind="ExternalInput").ap()
    f3in = nc.dram_tensor("f3in", [n_chunks, 128, 5, 3, PXW], bf,
                          kind="ExternalInput").ap()
    spin = nc.dram_tensor("spin", [n_chunks, 128, 4, WQ], bf,
                          kind="ExternalInput").ap()
    selin = nc.dram_tensor("selin", [128, MW], bf, kind="ExternalInput").ap()
    idin = nc.dram_tensor("idin", [128, 128], bf, kind="ExternalInput").ap()
    out = nc.dram_tensor("out", [n_chunks, 128, 3, WQ], f32,
                         kind="ExternalOutput").ap()

    with tile.TileContext(nc) as tc:
        with (
            tc.tile_pool(name="consts", bufs=1) as consts,
            tc.tile_pool(name="fload", bufs=1) as fload,
            tc.tile_pool(name="pxload", bufs=1) as pxload,
            tc.tile_pool(name="dpool", bufs=2) as dpool,
            tc.tile_pool(name="prpool", bufs=3) as prpool,
            tc.tile_pool(name="wpool", bufs=3) as wpool,
            tc.tile_pool(name="opool", bufs=1) as opool,
            tc.tile_pool(name="pspool", bufs=4, space="PSUM") as pspool,
            tc.tile_pool(name="psacc", bufs=1, space="PSUM") as psacc,
        ):
            selt = consts.tile([128, MW], bf)
            nc.sync.dma_start(out=selt, in_=selin)
            identt = consts.tile([128, 128], bf)
            nc.sync.dma_start(out=identt, in_=idin)
            onest = consts.tile([128, WQ], bf)
            nc.vector.memset(onest, 1.0)

            def emit_subsq(ft, ip):
                """Window diff (DVE) + square (ACT) for tap pair ip."""
                dy, dx = POS[ip]
                y0 = -3 * dy
                x0w = min(0, -3 * dx)
                wy = NY + 3 * dy
                wx = WQ + 3 * abs(dx)
                dft = dpool.tile([128, D2H, D2W], bf, tag="dft")
                dv = dft[:, :wy, :wx]
                i0y, i0x = 6 + y0, 6 + x0w
                i1y, i1x = 6 + y0 + 3 * dy, 6 + x0w + 3 * dx
                in0 = ft[:, i0y:i0y + wy, i0x:i0x + wx]
                in1 = ft[:, i1y:i1y + wy, i1x:i1x + wx]
                # GPSIMD is useless here: its SBUF port is an exclusive
                # lock shared with DVE, so GPSIMD tensor ops stall DVE.
                nc.vector.tensor_sub(out=dv, in0=in0, in1=in1)
                nc.scalar.activation(out=dv, in_=dv, func=AF.Square)
                return dft

            def emit_prods(pct, dft, ip):
                """pc * d2 for both signs of tap pair ip (DVE)."""
                dy, dx = POS[ip]
                prods = {}
                for sgn in (1, -1):
                    if sgn > 0:
                        ry, rx = 3 * dy, max(0, 3 * dx)
                    else:
                        ry, rx = 0, max(0, -3 * dx)
                    d2v = dft[:, ry:ry + NY, rx:rx + WQ]
                    prodt = prpool.tile([128, NY, WQ], bf, tag="prod",
                                        name=f"prod_{sgn}")
                    nc.vector.tensor_mul(out=prodt, in0=pct, in1=d2v)
                    prods[sgn] = prodt
                return prods

            def emit_yloop(spt, prods, ip):
                """Channel-reduce matmuls + spatial log-weights (PE).
                Consumes prods[+1] fully before prods[-1] so the + buffer
                frees at the halfway point for the next pair's muls."""
                dy, dx = POS[ip]
                lws = {}
                for sgn in (1, -1):
                    lws[sgn] = pspool.tile([128, WQ], f32, tag="lw",
                                           name=f"lw_{sgn}")
                for sgn in (1, -1):
                    for y in range(NY):
                        sy = S0 - (16 * (y // 4) + (y % 4))
                        nc.tensor.matmul(
                            out=lws[sgn],
                            lhsT=selt[:, sy:sy + 128],
                            rhs=prods[sgn][:, y, :],
                            start=(y == 0), stop=False,
                        )
                terms = _sp_terms(dy, dx)
                for it, ch in enumerate(terms):
                    for sgn in (1, -1):
                        nc.tensor.matmul(
                            out=lws[sgn], lhsT=identt, rhs=spt[:, ch, :],
                            start=False, stop=(it == len(terms) - 1),
                        )
                return lws

            def emit_exps(lws, wfts, ip):
                for sgn in (1, -1):
                    wft = wpool.tile([128, WQ], bf, tag=f"wft{ip}{sgn}",
                                     name=f"wft_{ip}_{sgn}", bufs=1)
                    nc.scalar.activation(out=wft, in_=lws[sgn], func=AF.Exp)
                    wfts[(ip, sgn)] = wft

            def emit_batch(f3t, wfts, pnums, pdent):
                """Center seeds + t3 muls (DVE) with identity-stationary
                accumulation matmuls (PE) trailing behind."""
                for ch in range(3):
                    nc.tensor.matmul(out=pnums[ch], lhsT=identt,
                                     rhs=f3t[:, 2, ch, 6:6 + WQ],
                                     start=True, stop=False)
                nc.tensor.matmul(out=pdent, lhsT=identt, rhs=onest,
                                 start=True, stop=False)
                for ip, (dy, dx) in enumerate(POS):
                    last = ip == len(POS) - 1
                    for sgn in (1, -1):
                        ddy, ddx = sgn * dy, sgn * dx
                        wft = wfts[(ip, sgn)]
                        t3 = wpool.tile([128, 3, WQ], bf, tag="t3",
                                        name=f"t3_{sgn}")
                        nc.vector.tensor_mul(
                            out=t3,
                            in0=bcast_mid(wft[:], 3),
                            in1=f3t[:, 2 + ddy, :,
                                    6 + 3 * ddx:6 + 3 * ddx + WQ],
                        )
                        stop = last and sgn == -1
                        for ch in range(3):
                            nc.tensor.matmul(out=pnums[ch], lhsT=identt,
                                             rhs=t3[:, ch, :],
                                             start=False, stop=stop)
                        nc.tensor.matmul(out=pdent, lhsT=identt, rhs=wft,
                                         start=False, stop=stop)

            prev_finals = None
            for j in range(n_chunks):
                ft = fload.tile([128, FH, FW], bf, tag="ft")
                pct = fload.tile([128, NY, WQ], bf, tag="pct")
                f3t = pxload.tile([128, 5, 3, PXW], bf, tag="f3t")
                spt = pxload.tile([128, 4, WQ], bf, tag="spt")
                nc.sync.dma_start(out=ft, in_=fin[j])
                nc.sync.dma_start(out=pct, in_=pcin[j])
                nc.sync.dma_start(out=f3t, in_=f3in[j])
                nc.sync.dma_start(out=spt, in_=spin[j])

                pnums = [psacc.tile([128, WQ], f32, tag=f"pnum{ch}",
                                    name=f"pnum{ch}") for ch in range(3)]
                pdent = psacc.tile([128, WQ], f32, tag="pden")
                wfts = {}

                # software pipeline, two pairs deep: the serial chain
                # sub -> square -> prods spans DVE -> ACT -> DVE, so the sub
                # runs TWO pairs ahead.  Then square k+1 (ACT) overlaps
                # prods k (DVE) and the DVE never idles on the ACT square.
                dfts = {0: emit_subsq(ft, 0), 1: emit_subsq(ft, 1)}
                prods = {0: emit_prods(pct, dfts.pop(0), 0)}
                for k in range(len(POS)):
                    lws = emit_yloop(spt, prods.pop(k), k)
                    if k + 2 < len(POS):
                        dfts[k + 2] = emit_subsq(ft, k + 2)
                    if k + 1 < len(POS):
                        prods[k + 1] = emit_prods(pct, dfts.pop(k + 1), k + 1)
                    emit_exps(lws, wfts, k)
                    if k == 0 and prev_finals is not None:
                        prev_finals()
                        prev_finals = None
                emit_batch(f3t, wfts, pnums, pdent)

                def make_finals(j=j, pnums=pnums, pdent=pdent):
                    def finals():
                        rden = wpool.tile([128, WQ], f32, tag="rden", bufs=1)
                        nc.vector.reciprocal(out=rden, in_=pdent)
                        osb = opool.tile([128, 3, WQ], f32, tag="osb")
                        for ch in range(3):
                            nc.vector.tensor_mul(out=osb[:, ch, :],
                                                 in0=pnums[ch], in1=rden)
                        nc.sync.dma_start(out=out[j], in_=osb)
                    return finals
                prev_finals = make_finals()
            prev_finals()

    nc.compile()
    return nc


def prep_inputs(input, coeffs, n_chunks=NCH):
    """Build per-core in_maps (list of 8 dicts of numpy arrays)."""
    inp = np.asarray(input, np.float32)
    f = inp[:, :C]                      # [2,32,720,1280]
    scale = inp[:, C:]                  # [2,34,720,1280]
    k = np.exp(np.asarray(coeffs, np.float32).reshape(-1))   # [34]
    sp = np.logaddexp(0.0, scale)
    params = -(k[None, :, None, None] * sp)
    pc = params[:, :C]
    psy = params[:, C]                  # [2,720,1280]
    psx = params[:, C + 1]

    # padded f: rows/cols shifted by +6
    fp = np.full((B, C, H + 12, W + 12), PADV, np.float32)
    fp[:, :, 6:6 + H, 6:6 + W] = f
    # padded first-3-channel f for the pixel stage: shifted by +6
    f3p = np.full((B, 3, H + 12, W + 12), PADV, np.float32)
    f3p[:, :, 6:6 + H, 6:6 + W] = f[:, :3]

    # selection master matrix: sel[(g,c), v] = 1 iff v == S0 + 4g
    sel = np.zeros((128, MW), np.float32)
    for g in range(NG):
        sel[32 * g:32 * (g + 1), S0 + 4 * g] = 1.0
    ident = np.eye(128, dtype=np.float32)

    # row-gather index with holes -> clamp to row 0 and zero later
    prow = np.where(REAL, PPERM, 0)

    in_maps = []
    for b in range(B):
        for q in range(4):
            x0 = WQ * q
            fpb = fp[b, :, :, x0:x0 + FW]          # [32, 732, 332]
            s = fpb.strides
            fin = np.lib.stride_tricks.as_strided(
                fpb, shape=(n_chunks, NG, C, FH, FW),
                strides=(CH * s[1], NY * s[1], s[0], s[1], s[2]),
            ).reshape(n_chunks, 128, FH, FW)

            pcb = pc[b, :, :, x0:x0 + WQ]          # [32, 720, 320]
            s = pcb.strides
            pcin = np.lib.stride_tricks.as_strided(
                pcb, shape=(n_chunks, NG, C, NY, WQ),
                strides=(CH * s[1], NY * s[1], s[0], s[1], s[2]),
            ).reshape(n_chunks, 128, NY, WQ)

            # f3in[j, d, p, c, xx] = f3p[b, c, 120j + prow[p] + 3(d-2) + 6, x0+xx]
            j_idx = np.arange(n_chunks)[:, None, None]
            d_idx = np.arange(5)[None, :, None]
            p_idx = prow[None, None, :]
            rows = CH * j_idx + p_idx + 3 * (d_idx - 2) + 6   # [j, d, p]
            f3in = f3p[b][:, rows, x0:x0 + PXW]               # [3, j, d, p, PXW]
            # -> [j, p, d, c, x] to match SBUF tile [128, 5, 3, PXW]
            f3in = np.ascontiguousarray(f3in.transpose(1, 3, 2, 0, 4))
            f3in[:, ~REAL] = 0.0

            # spin[j, p, m, xx]: spatial coefficient maps in pixel layout,
            # channels [psy, psx, 4*psy, 4*psx]
            rows2 = CH * np.arange(n_chunks)[:, None] + prow[None, :]  # [j, p]
            spy = psy[b][rows2, x0:x0 + WQ]                   # [j, p, WQ]
            spx = psx[b][rows2, x0:x0 + WQ]
            spin = np.stack([spy, spx, 4.0 * spy, 4.0 * spx], axis=2)
            spin[:, ~REAL] = 0.0

            in_maps.append({
                "fin": fin.astype(BF16),
                "pcin": pcin.astype(BF16),
                "f3in": f3in.astype(BF16),
                "spin": np.ascontiguousarray(spin).astype(BF16),
                "selin": sel.astype(BF16),
                "idin": ident.astype(BF16),
            })
    return in_maps


def assemble_output(results, n_chunks=NCH):
    outf = np.empty((B, 3, H, W), np.float32)
    i = 0
    for b in range(B):
        for q in range(4):
            x0 = WQ * q
            o = np.asarray(results[i]["out"], np.float32)  # [j, 128, 3, WQ]
            for j in range(n_chunks):
                # fancy-index on axis 2 with slice on axis 1 -> result axes
                # are (row, c, x), matching o[j, REAL] directly
                outf[b, :, CH * j + PPERM[REAL], x0:x0 + WQ] = o[j, REAL]
            i += 1
    return outf


_NC_CACHE = {}


def kernel(input, coeffs, kernel_size=5, dilation=3, dynamic_size=3):
    assert int(kernel_size) == 5 and int(dilation) == 3
    assert int(dynamic_size) == 3
    from concourse import bass_utils

    if "nc" not in _NC_CACHE:
        _NC_CACHE["nc"] = build_nc(NCH)
    nc = _NC_CACHE["nc"]
    in_maps = prep_inputs(input, coeffs, NCH)
    res = bass_utils.run_bass_kernel_spmd(nc, in_maps,
                                          core_ids=list(range(NCORE)))
    return assemble_output(res.results, NCH)


# revision 9
# speedup vs baseline: 1.4856x; 1.0003x over previous
"""Trainium2 Bass kernel for BetterPixelBilateralFilter2.

Problem: 5x5 dilated (dilation=3) bilateral filter over [B=2, C=32, 720, 1280]
with per-pixel range coefficients pc = -exp(coeffs)*softplus(scale) and
per-pixel spatial coefficients psy/psx.  Output = first 3 filtered channels.

Sharding: 8 cores = batch(2) x W-quarter(4).  Each core handles a full-height
[720, 320] slab of one batch image.

Device layout (per core), 6 chunks of 120 rows (= 4 subchunks x 30):
  - channel stage: partitions = (subchunk g, channel c) = 4x32; free = (y, x).
    All tap shifts are free-dim view offsets.  Per tap-pair: diff (DVE),
    square (ACT), mul-by-pc (DVE).
  - channel reduce: per y-row, a matmul with a shifted view of a constant
    selection matrix (lhsT[:, p] = 1 iff p == pixel_partition(g, y))
    accumulates into ONE PSUM [128, 320] tile per tap that lands directly in
    pixel layout: partition p <-> row y = 4*(p//16) + p%4, g = (p%16)//4.
    The spatial log-weights psy*dy^2 + psx*dx^2 are added into the same PSUM
    by identity-stationary matmuls streaming host-prepared psy/psx maps.
  - pixel stage: exp straight from PSUM (ACT) -> w; t3 = w * f3 neighbors
    (DVE, single mul per tap); num (3ch) and den accumulate in PSUM via
    identity-stationary matmuls (TensorE), center tap seeds the group.
    Finals: den+1 (center weight), reciprocal, 3 muls, DMA out.
  - 8 hole partitions (y>=30 slots) carry zeros and are dropped on the host.

Border handling: host pads f with 1e4; (f - 1e4)^2 * pc <= -3e4 so exp
underflows to exactly 0 -- out-of-image taps contribute nothing.
"""

import numpy as np
import ml_dtypes

BF16 = ml_dtypes.bfloat16
PADV = 1.0e4

B, C, H, W = 2, 32, 720, 1280
NCORE = 8
WQ = 320           # x-quarter width per core
CH = 120           # rows per chunk
NG = 4             # y-subchunks per chunk
NY = 30            # rows per subchunk
NCH = H // CH      # 6 chunks
FH, FW = NY + 12, WQ + 12      # f-tile window 42 x 332 (+-6 halo)
D2H, D2W = 36, 326             # max diff-window (30+6, 320+6)
PXW = WQ + 12                  # f3 x-window 332
S0 = 113                       # selection-matrix center column
MW = S0 + 128                  # master selection matrix width

# positive tap offsets (dy,dx); each also covers its negation
POS = [(0, 1), (0, 2),
       (1, -2), (1, -1), (1, 0), (1, 1), (1, 2),
       (2, -2), (2, -1), (2, 0), (2, 1), (2, 2)]


def _sp_terms(dy, dx):
    """Spatial-map channels to add for this tap pair: dy^2*psy + dx^2*psx,
    using spyx channels [psy, psx, 4*psy, 4*psx]."""
    terms = []
    if dy * dy == 1:
        terms.append(0)
    elif dy * dy == 4:
        terms.append(2)
    if dx * dx == 1:
        terms.append(1)
    elif dx * dx == 4:
        terms.append(3)
    return terms


def _pixel_perm():
    """pperm[p] = chunk-local row (30*g + y_sub) for real partitions, -1 holes."""
    pperm = np.full(128, -1, np.int64)
    for y in range(NY):
        h, r = divmod(y, 4)
        for g in range(NG):
            pperm[16 * h + 4 * g + r] = NY * g + y
    return pperm


PPERM = _pixel_perm()          # [128], -1 at 8 hole slots
REAL = PPERM >= 0


def build_nc(n_chunks=NCH):
    import concourse.bacc as bacc
    import concourse.bass as bass
    import concourse.tile as tile
    from concourse import mybir

    def bcast_mid(a, n):
        """[P, X] view -> [P, n, X] with a stride-0 middle dim."""
        return bass.AP(tensor=a.tensor, offset=a.offset,
                       ap=[a.ap[0], [0, n], a.ap[1]])

    bf = mybir.dt.bfloat16
    f32 = mybir.dt.float32
    AF = mybir.ActivationFunctionType
    OP = mybir.AluOpType

    nc = bacc.Bacc("TRN2", num_devices=NCORE, debug=False)
    fin = nc.dram_tensor("fin", [n_chunks, 128, FH, FW], bf,
                         kind="ExternalInput").ap()
    pcin = nc.dram_tensor("pcin", [n_chunks, 128, NY, WQ], bf,
                          kind="ExternalInput").ap()
    f3in = nc.dram_tensor("f3in", [n_chunks, 128, 5, 3, PXW], bf,
                          kind="ExternalInput").ap()
    spin = nc.dram_tensor("spin", [n_chunks, 128, 4, WQ], bf,
                          kind="ExternalInput").ap()
    selin = nc.dram_tensor("selin", [128, MW], bf, kind="ExternalInput").ap()
    idin = nc.dram_tensor("idin", [128, 128], bf, kind="ExternalInput").ap()
    out = nc.dram_tensor("out", [n_chunks, 128, 3, WQ], f32,
                         kind="ExternalOutput").ap()

    with tile.TileContext(nc) as tc:
        with (
            tc.tile_pool(name="consts", bufs=1) as consts,
            tc.tile_pool(name="fload", bufs=1) as fload,
            tc.tile_pool(name="pxload", bufs=1) as pxload,
            tc.tile_pool(name="dpool", bufs=2) as dpool,
            tc.tile_pool(name="prpool", bufs=3) as prpool,
            tc.tile_pool(name="wpool", bufs=3) as wpool,
            tc.tile_pool(name="opool", bufs=1) as opool,
            tc.tile_pool(name="pspool", bufs=4, space="PSUM") as pspool,
            tc.tile_pool(name="psacc", bufs=1, space="PSUM") as psacc,
        ):
            selt = consts.tile([128, MW], bf)
            nc.sync.dma_start(out=selt, in_=selin)
            identt = consts.tile([128, 128], bf)
            nc.sync.dma_start(out=identt, in_=idin)
            onest = consts.tile([128, WQ], bf)
            nc.vector.memset(onest, 1.0)

            def emit_subsq(ft, ip):
                """Window diff (DVE) + square (ACT) for tap pair ip."""
                dy, dx = POS[ip]
                y0 = -3 * dy
                x0w = min(0, -3 * dx)
                wy = NY + 3 * dy
                wx = WQ + 3 * abs(dx)
                dft = dpool.tile([128, D2H, D2W], bf, tag="dft")
                dv = dft[:, :wy, :wx]
                i0y, i0x = 6 + y0, 6 + x0w
                i1y, i1x = 6 + y0 + 3 * dy, 6 + x0w + 3 * dx
                in0 = ft[:, i0y:i0y + wy, i0x:i0x + wx]
                in1 = ft[:, i1y:i1y + wy, i1x:i1x + wx]
                # GPSIMD is useless here: its SBUF port is an exclusive
                # lock shared with DVE, so GPSIMD tensor ops stall DVE.
                nc.vector.tensor_sub(out=dv, in0=in0, in1=in1)
                nc.scalar.activation(out=dv, in_=dv, func=AF.Square)
                return dft

            def emit_prods(pct, dft, ip):
                """pc * d2 for both signs of tap pair ip (DVE)."""
                dy, dx = POS[ip]
                prods = {}
                for sgn in (1, -1):
                    if sgn > 0:
                        ry, rx = 3 * dy, max(0, 3 * dx)
                    else:
                        ry, rx = 0, max(0, -3 * dx)
                    d2v = dft[:, ry:ry + NY, rx:rx + WQ]
                    prodt = prpool.tile([128, NY, WQ], bf, tag="prod",
                                        name=f"prod_{sgn}")
                    nc.vector.tensor_mul(out=prodt, in0=pct, in1=d2v)
                    prods[sgn] = prodt
                return prods

            def emit_yloop(spt, prods, ip):
                """Channel-reduce matmuls + spatial log-weights (PE).
                Consumes prods[+1] fully before prods[-1] so the + buffer
                frees at the halfway point for the next pair's muls."""
                dy, dx = POS[ip]
                lws = {}
                for sgn in (1, -1):
                    lws[sgn] = pspool.tile([128, WQ], f32, tag="lw",
                                           name=f"lw_{sgn}")
                for sgn in (1, -1):
                    for y in range(NY):
                        sy = S0 - (16 * (y // 4) + (y % 4))
                        nc.tensor.matmul(
                            out=lws[sgn],
                            lhsT=selt[:, sy:sy + 128],
                            rhs=prods[sgn][:, y, :],
                            start=(y == 0), stop=False,
                        )
                terms = _sp_terms(dy, dx)
                for it, ch in enumerate(terms):
                    for sgn in (1, -1):
                        nc.tensor.matmul(
                            out=lws[sgn], lhsT=identt, rhs=spt[:, ch, :],
                            start=False, stop=(it == len(terms) - 1),
                        )
                return lws

            def emit_exps(lws, wfts, ip):
                for sgn in (1, -1):
                    wft = wpool.tile([128, WQ], bf, tag=f"wft{ip}{sgn}",
                                     name=f"wft_{ip}_{sgn}", bufs=1)
                    nc.scalar.activation(out=wft, in_=lws[sgn], func=AF.Exp)
                    wfts[(ip, sgn)] = wft

            def emit_batch(f3t, wfts, pnums, pdent):
                """Center seeds + t3 muls (DVE) with identity-stationary
                accumulation matmuls (PE) trailing behind."""
                for ch in range(3):
                    nc.tensor.matmul(out=pnums[ch], lhsT=identt,
                                     rhs=f3t[:, 2, ch, 6:6 + WQ],
                                     start=True, stop=False)
                nc.tensor.matmul(out=pdent, lhsT=identt, rhs=onest,
                                 start=True, stop=False)
                for ip, (dy, dx) in enumerate(POS):
                    last = ip == len(POS) - 1
                    for sgn in (1, -1):
                        ddy, ddx = sgn * dy, sgn * dx
                        wft = wfts[(ip, sgn)]
                        t3 = wpool.tile([128, 3, WQ], bf, tag="t3",
                                        name=f"t3_{sgn}")
                        nc.vector.tensor_mul(
                            out=t3,
                            in0=bcast_mid(wft[:], 3),
                            in1=f3t[:, 2 + ddy, :,
                                    6 + 3 * ddx:6 + 3 * ddx + WQ],
                        )
                        stop = last and sgn == -1
                        for ch in range(3):
                            nc.tensor.matmul(out=pnums[ch], lhsT=identt,
                                             rhs=t3[:, ch, :],
                                             start=False, stop=stop)
                        nc.tensor.matmul(out=pdent, lhsT=identt, rhs=wft,
                                         start=False, stop=stop)

            prev_finals = None
            pending_batch = None
            for j in range(n_chunks):
                ft = fload.tile([128, FH, FW], bf, tag="ft")
                pct = fload.tile([128, NY, WQ], bf, tag="pct", bufs=2)
                f3t = pxload.tile([128, 5, 3, PXW], bf, tag="f3t")
                spt = pxload.tile([128, 4, WQ], bf, tag="spt")
                nc.sync.dma_start(out=ft, in_=fin[j])
                nc.sync.dma_start(out=pct, in_=pcin[j])
                nc.sync.dma_start(out=f3t, in_=f3in[j])
                nc.sync.dma_start(out=spt, in_=spin[j])

                pnums = [psacc.tile([128, WQ], f32, tag=f"pnum{ch}",
                                    name=f"pnum{ch}") for ch in range(3)]
                pdent = psacc.tile([128, WQ], f32, tag="pden")
                wfts = {}

                # software pipeline, two pairs deep: the serial chain
                # sub -> square -> prods spans DVE -> ACT -> DVE, so the sub
                # runs TWO pairs ahead.  Then square k+1 (ACT) overlaps
                # prods k (DVE) and the DVE never idles on the ACT square.
                # This chunk's first two subs are emitted BEFORE the previous
                # chunk's batch phase so their squares hide under its t3 muls.
                dfts = {0: emit_subsq(ft, 0), 1: emit_subsq(ft, 1)}
                if pending_batch is not None:
                    pending_batch()
                prods = {0: emit_prods(pct, dfts.pop(0), 0)}
                for k in range(len(POS)):
                    lws = emit_yloop(spt, prods.pop(k), k)
                    if k + 2 < len(POS):
                        dfts[k + 2] = emit_subsq(ft, k + 2)
                    if k + 1 < len(POS):
                        prods[k + 1] = emit_prods(pct, dfts.pop(k + 1), k + 1)
                    emit_exps(lws, wfts, k)
                    if k == 0 and prev_finals is not None:
                        prev_finals()
                        prev_finals = None

                def make_batch(f3t=f3t, wfts=wfts, pnums=pnums, pdent=pdent):
                    return lambda: emit_batch(f3t, wfts, pnums, pdent)
                pending_batch = make_batch()

                def make_finals(j=j, pnums=pnums, pdent=pdent):
                    def finals():
                        rden = wpool.tile([128, WQ], f32, tag="rden", bufs=1)
                        nc.vector.reciprocal(out=rden, in_=pdent)
                        osb = opool.tile([128, 3, WQ], f32, tag="osb")
                        for ch in range(3):
                            nc.vector.tensor_mul(out=osb[:, ch, :],
                                                 in0=pnums[ch], in1=rden)
                        nc.sync.dma_start(out=out[j], in_=osb)
                    return finals
                prev_finals = make_finals()
            pending_batch()
            prev_finals()

    nc.compile()
    return nc


def prep_inputs(input, coeffs, n_chunks=NCH):
    """Build per-core in_maps (list of 8 dicts of numpy arrays)."""
    inp = np.asarray(input, np.float32)
    f = inp[:, :C]                      # [2,32,720,1280]
    scale = inp[:, C:]                  # [2,34,720,1280]
    k = np.exp(np.asarray(coeffs, np.float32).reshape(-1))   # [34]
    sp = np.logaddexp(0.0, scale)
    params = -(k[None, :, None, None] * sp)
    pc = params[:, :C]
    psy = params[:, C]                  # [2,720,1280]
    psx = params[:, C + 1]

    # padded f: rows/cols shifted by +6
    fp = np.full((B, C, H + 12, W + 12), PADV, np.float32)
    fp[:, :, 6:6 + H, 6:6 + W] = f
    # padded first-3-channel f for the pixel stage: shifted by +6
    f3p = np.full((B, 3, H + 12, W + 12), PADV, np.float32)
    f3p[:, :, 6:6 + H, 6:6 + W] = f[:, :3]

    # selection master matrix: sel[(g,c), v] = 1 iff v == S0 + 4g
    sel = np.zeros((128, MW), np.float32)
    for g in range(NG):
        sel[32 * g:32 * (g + 1), S0 + 4 * g] = 1.0
    ident = np.eye(128, dtype=np.float32)

    # row-gather index with holes -> clamp to row 0 and zero later
    prow = np.where(REAL, PPERM, 0)

    in_maps = []
    for b in range(B):
        for q in range(4):
            x0 = WQ * q
            fpb = fp[b, :, :, x0:x0 + FW]          # [32, 732, 332]
            s = fpb.strides
            fin = np.lib.stride_tricks.as_strided(
                fpb, shape=(n_chunks, NG, C, FH, FW),
                strides=(CH * s[1], NY * s[1], s[0], s[1], s[2]),
            ).reshape(n_chunks, 128, FH, FW)

            pcb = pc[b, :, :, x0:x0 + WQ]          # [32, 720, 320]
            s = pcb.strides
            pcin = np.lib.stride_tricks.as_strided(
                pcb, shape=(n_chunks, NG, C, NY, WQ),
                strides=(CH * s[1], NY * s[1], s[0], s[1], s[2]),
            ).reshape(n_chunks, 128, NY, WQ)

            # f3in[j, d, p, c, xx] = f3p[b, c, 120j + prow[p] + 3(d-2) + 6, x0+xx]
            j_idx = np.arange(n_chunks)[:, None, None]
            d_idx = np.arange(5)[None, :, None]
            p_idx = prow[None, None, :]
            rows = CH * j_idx + p_idx + 3 * (d_idx - 2) + 6   # [j, d, p]
            f3in = f3p[b][:, rows, x0:x0 + PXW]               # [3, j, d, p, PXW]
            # -> [j, p, d, c, x] to match SBUF tile [128, 5, 3, PXW]
            f3in = np.ascontiguousarray(f3in.transpose(1, 3, 2, 0, 4))
            f3in[:, ~REAL] = 0.0

            # spin[j, p, m, xx]: spatial coefficient maps in pixel layout,
            # channels [psy, psx, 4*psy, 4*psx]
            rows2 = CH * np.arange(n_chunks)[:, None] + prow[None, :]  # [j, p]
            spy = psy[b][rows2, x0:x0 + WQ]                   # [j, p, WQ]
            spx = psx[b][rows2, x0:x0 + WQ]
            spin = np.stack([spy, spx, 4.0 * spy, 4.0 * spx], axis=2)
            spin[:, ~REAL] = 0.0

            in_maps.append({
                "fin": fin.astype(BF16),
                "pcin": pcin.astype(BF16),
                "f3in": f3in.astype(BF16),
                "spin": np.ascontiguousarray(spin).astype(BF16),
                "selin": sel.astype(BF16),
                "idin": ident.astype(BF16),
            })
    return in_maps


def assemble_output(results, n_chunks=NCH):
    outf = np.empty((B, 3, H, W), np.float32)
    i = 0
    for b in range(B):
        for q in range(4):
            x0 = WQ * q
            o = np.asarray(results[i]["out"], np.float32)  # [j, 128, 3, WQ]
            for j in range(n_chunks):
                # fancy-index on axis 2 with slice on axis 1 -> result axes
                # are (row, c, x), matching o[j, REAL] directly
                outf[b, :, CH * j + PPERM[REAL], x0:x0 + WQ] = o[j, REAL]
            i += 1
    return outf


_NC_CACHE = {}


def kernel(input, coeffs, kernel_size=5, dilation=3, dynamic_size=3):
    assert int(kernel_size) == 5 and int(dilation) == 3
    assert int(dynamic_size) == 3
    from concourse import bass_utils

    if "nc" not in _NC_CACHE:
        _NC_CACHE["nc"] = build_nc(NCH)
    nc = _NC_CACHE["nc"]
    in_maps = prep_inputs(input, coeffs, NCH)
    res = bass_utils.run_bass_kernel_spmd(nc, in_maps,
                                          core_ids=list(range(NCORE)))
    return assemble_output(res.results, NCH)


# revision 10
# speedup vs baseline: 1.4879x; 1.0015x over previous
"""Trainium2 Bass kernel for BetterPixelBilateralFilter2.

Problem: 5x5 dilated (dilation=3) bilateral filter over [B=2, C=32, 720, 1280]
with per-pixel range coefficients pc = -exp(coeffs)*softplus(scale) and
per-pixel spatial coefficients psy/psx.  Output = first 3 filtered channels.

Sharding: 8 cores = batch(2) x W-quarter(4).  Each core handles a full-height
[720, 320] slab of one batch image.

Device layout (per core), 6 chunks of 120 rows (= 4 subchunks x 30):
  - channel stage: partitions = (subchunk g, channel c) = 4x32; free = (y, x).
    All tap shifts are free-dim view offsets.  Per tap-pair: diff (DVE),
    square (ACT), mul-by-pc (DVE).
  - channel reduce: per y-row, a matmul with a shifted view of a constant
    selection matrix (lhsT[:, p] = 1 iff p == pixel_partition(g, y))
    accumulates into ONE PSUM [128, 320] tile per tap that lands directly in
    pixel layout: partition p <-> row y = 4*(p//16) + p%4, g = (p%16)//4.
    The spatial log-weights psy*dy^2 + psx*dx^2 are added into the same PSUM
    by identity-stationary matmuls streaming host-prepared psy/psx maps.
  - pixel stage: exp straight from PSUM (ACT) -> w; t3 = w * f3 neighbors
    (DVE, single mul per tap); num (3ch) and den accumulate in PSUM via
    identity-stationary matmuls (TensorE), center tap seeds the group.
    Finals: den+1 (center weight), reciprocal, 3 muls, DMA out.
  - 8 hole partitions (y>=30 slots) carry zeros and are dropped on the host.

Border handling: host pads f with 1e4; (f - 1e4)^2 * pc <= -3e4 so exp
underflows to exactly 0 -- out-of-image taps contribute nothing.
"""

import numpy as np
import ml_dtypes

BF16 = ml_dtypes.bfloat16
PADV = 1.0e4

B, C, H, W = 2, 32, 720, 1280
NCORE = 8
WQ = 320           # x-quarter width per core
CH = 120           # rows per chunk
NG = 4             # y-subchunks per chunk
NY = 30            # rows per subchunk
NCH = H // CH      # 6 chunks
FH, FW = NY + 12, WQ + 12      # f-tile window 42 x 332 (+-6 halo)
D2H, D2W = 36, 326             # max diff-window (30+6, 320+6)
PXW = WQ + 12                  # f3 x-window 332
S0 = 113                       # selection-matrix center column
MW = S0 + 128                  # master selection matrix width

# positive tap offsets (dy,dx); each also covers its negation
POS = [(0, 1), (0, 2),
       (1, -2), (1, -1), (1, 0), (1, 1), (1, 2),
       (2, -2), (2, -1), (2, 0), (2, 1), (2, 2)]


def _sp_terms(dy, dx):
    """Spatial-map channels to add for this tap pair: dy^2*psy + dx^2*psx,
    using spyx channels [psy, psx, 4*psy, 4*psx]."""
    terms = []
    if dy * dy == 1:
        terms.append(0)
    elif dy * dy == 4:
        terms.append(2)
    if dx * dx == 1:
        terms.append(1)
    elif dx * dx == 4:
        terms.append(3)
    return terms


def _pixel_perm():
    """pperm[p] = chunk-local row (30*g + y_sub) for real partitions, -1 holes."""
    pperm = np.full(128, -1, np.int64)
    for y in range(NY):
        h, r = divmod(y, 4)
        for g in range(NG):
            pperm[16 * h + 4 * g + r] = NY * g + y
    return pperm


PPERM = _pixel_perm()          # [128], -1 at 8 hole slots
REAL = PPERM >= 0


def build_nc(n_chunks=NCH):
    import concourse.bacc as bacc
    import concourse.bass as bass
    import concourse.tile as tile
    from concourse import mybir

    def bcast_mid(a, n):
        """[P, X] view -> [P, n, X] with a stride-0 middle dim."""
        return bass.AP(tensor=a.tensor, offset=a.offset,
                       ap=[a.ap[0], [0, n], a.ap[1]])

    bf = mybir.dt.bfloat16
    f32 = mybir.dt.float32
    AF = mybir.ActivationFunctionType
    OP = mybir.AluOpType

    nc = bacc.Bacc("TRN2", num_devices=NCORE, debug=False)
    fin = nc.dram_tensor("fin", [n_chunks, 128, FH, FW], bf,
                         kind="ExternalInput").ap()
    pcin = nc.dram_tensor("pcin", [n_chunks, 128, NY, WQ], bf,
                          kind="ExternalInput").ap()
    f3in = nc.dram_tensor("f3in", [n_chunks, 128, 5, 3, PXW], bf,
                          kind="ExternalInput").ap()
    spin = nc.dram_tensor("spin", [n_chunks, 128, 4, WQ], bf,
                          kind="ExternalInput").ap()
    selin = nc.dram_tensor("selin", [128, MW], bf, kind="ExternalInput").ap()
    idin = nc.dram_tensor("idin", [128, 128], bf, kind="ExternalInput").ap()
    out = nc.dram_tensor("out", [n_chunks, 128, 3, WQ], f32,
                         kind="ExternalOutput").ap()

    with tile.TileContext(nc) as tc:
        with (
            tc.tile_pool(name="consts", bufs=1) as consts,
            tc.tile_pool(name="fload", bufs=1) as fload,
            tc.tile_pool(name="pxload", bufs=1) as pxload,
            tc.tile_pool(name="dpool", bufs=2) as dpool,
            tc.tile_pool(name="prpool", bufs=3) as prpool,
            tc.tile_pool(name="wpool", bufs=3) as wpool,
            tc.tile_pool(name="opool", bufs=1) as opool,
            tc.tile_pool(name="pspool", bufs=4, space="PSUM") as pspool,
            tc.tile_pool(name="psacc", bufs=1, space="PSUM") as psacc,
        ):
            selt = consts.tile([128, MW], bf)
            nc.sync.dma_start(out=selt, in_=selin)
            identt = consts.tile([128, 128], bf)
            nc.sync.dma_start(out=identt, in_=idin)
            onest = consts.tile([128, WQ], bf)
            nc.vector.memset(onest, 1.0)

            def emit_subsq(ft, ip):
                """Window diff (DVE) + square (ACT) for tap pair ip."""
                dy, dx = POS[ip]
                y0 = -3 * dy
                x0w = min(0, -3 * dx)
                wy = NY + 3 * dy
                wx = WQ + 3 * abs(dx)
                dft = dpool.tile([128, D2H, D2W], bf, tag="dft")
                dv = dft[:, :wy, :wx]
                i0y, i0x = 6 + y0, 6 + x0w
                i1y, i1x = 6 + y0 + 3 * dy, 6 + x0w + 3 * dx
                in0 = ft[:, i0y:i0y + wy, i0x:i0x + wx]
                in1 = ft[:, i1y:i1y + wy, i1x:i1x + wx]
                # GPSIMD is useless here: its SBUF port is an exclusive
                # lock shared with DVE, so GPSIMD tensor ops stall DVE.
                nc.vector.tensor_sub(out=dv, in0=in0, in1=in1)
                nc.scalar.activation(out=dv, in_=dv, func=AF.Square)
                return dft

            def emit_prods(pct, dft, ip):
                """pc * d2 for both signs of tap pair ip (DVE)."""
                dy, dx = POS[ip]
                prods = {}
                for sgn in (1, -1):
                    if sgn > 0:
                        ry, rx = 3 * dy, max(0, 3 * dx)
                    else:
                        ry, rx = 0, max(0, -3 * dx)
                    d2v = dft[:, ry:ry + NY, rx:rx + WQ]
                    prodt = prpool.tile([128, NY, WQ], bf, tag="prod",
                                        name=f"prod_{sgn}")
                    nc.vector.tensor_mul(out=prodt, in0=pct, in1=d2v)
                    prods[sgn] = prodt
                return prods

            def emit_yloop(spt, prods, ip):
                """Channel-reduce matmuls + spatial log-weights (PE).
                Consumes prods[+1] fully before prods[-1] so the + buffer
                frees at the halfway point for the next pair's muls."""
                dy, dx = POS[ip]
                lws = {}
                for sgn in (1, -1):
                    lws[sgn] = pspool.tile([128, WQ], f32, tag="lw",
                                           name=f"lw_{sgn}")
                for sgn in (1, -1):
                    for y in range(NY):
                        sy = S0 - (16 * (y // 4) + (y % 4))
                        nc.tensor.matmul(
                            out=lws[sgn],
                            lhsT=selt[:, sy:sy + 128],
                            rhs=prods[sgn][:, y, :],
                            start=(y == 0), stop=False,
                        )
                terms = _sp_terms(dy, dx)
                for it, ch in enumerate(terms):
                    for sgn in (1, -1):
                        nc.tensor.matmul(
                            out=lws[sgn], lhsT=identt, rhs=spt[:, ch, :],
                            start=False, stop=(it == len(terms) - 1),
                        )
                return lws

            def emit_exps(lws, wfts, ip):
                for sgn in (1, -1):
                    wft = wpool.tile([128, WQ], bf, tag=f"wft{ip}{sgn}",
                                     name=f"wft_{ip}_{sgn}", bufs=1)
                    nc.scalar.activation(out=wft, in_=lws[sgn], func=AF.Exp)
                    wfts[(ip, sgn)] = wft

            def emit_batch(f3t, wfts, pnums, pdent):
                """Center seeds + t3 muls (DVE) with identity-stationary
                accumulation matmuls (PE) trailing behind."""
                for ch in range(3):
                    nc.tensor.matmul(out=pnums[ch], lhsT=identt,
                                     rhs=f3t[:, 2, ch, 6:6 + WQ],
                                     start=True, stop=False)
                nc.tensor.matmul(out=pdent, lhsT=identt, rhs=onest,
                                 start=True, stop=False)
                for ip, (dy, dx) in enumerate(POS):
                    last = ip == len(POS) - 1
                    for sgn in (1, -1):
                        ddy, ddx = sgn * dy, sgn * dx
                        wft = wfts[(ip, sgn)]
                        t3 = wpool.tile([128, 3, WQ], bf, tag="t3",
                                        name=f"t3_{sgn}")
                        nc.vector.tensor_mul(
                            out=t3,
                            in0=bcast_mid(wft[:], 3),
                            in1=f3t[:, 2 + ddy, :,
                                    6 + 3 * ddx:6 + 3 * ddx + WQ],
                        )
                        stop = last and sgn == -1
                        for ch in range(3):
                            nc.tensor.matmul(out=pnums[ch], lhsT=identt,
                                             rhs=t3[:, ch, :],
                                             start=False, stop=stop)
                        nc.tensor.matmul(out=pdent, lhsT=identt, rhs=wft,
                                         start=False, stop=stop)

            prev_finals = None
            pending_batch = None
            for j in range(n_chunks):
                ft = fload.tile([128, FH, FW], bf, tag="ft")
                pct = fload.tile([128, NY, WQ], bf, tag="pct", bufs=2)
                f3t = pxload.tile([128, 5, 3, PXW], bf, tag="f3t")
                spt = pxload.tile([128, 4, WQ], bf, tag="spt")
                nc.sync.dma_start(out=ft, in_=fin[j])
                nc.sync.dma_start(out=pct, in_=pcin[j])
                nc.sync.dma_start(out=f3t, in_=f3in[j])
                nc.sync.dma_start(out=spt, in_=spin[j])

                pnums = [psacc.tile([128, WQ], f32, tag=f"pnum{ch}",
                                    name=f"pnum{ch}") for ch in range(3)]
                pdent = psacc.tile([128, WQ], f32, tag="pden")
                wfts = {}

                # software pipeline, two pairs deep: the serial chain
                # sub -> square -> prods spans DVE -> ACT -> DVE, so the sub
                # runs TWO pairs ahead.  Then square k+1 (ACT) overlaps
                # prods k (DVE) and the DVE never idles on the ACT square.
                # This chunk's first two subs are emitted BEFORE the previous
                # chunk's batch phase so their squares hide under its t3 muls.
                dfts = {0: emit_subsq(ft, 0), 1: emit_subsq(ft, 1)}
                if pending_batch is not None:
                    pending_batch()
                prods = {0: emit_prods(pct, dfts.pop(0), 0)}
                for k in range(len(POS)):
                    lws = emit_yloop(spt, prods.pop(k), k)
                    if k + 2 < len(POS):
                        dfts[k + 2] = emit_subsq(ft, k + 2)
                    if k + 1 < len(POS):
                        prods[k + 1] = emit_prods(pct, dfts.pop(k + 1), k + 1)
                    emit_exps(lws, wfts, k)
                    if k == 0 and prev_finals is not None:
                        prev_finals()
                        prev_finals = None

                def make_batch(f3t=f3t, wfts=wfts, pnums=pnums, pdent=pdent):
                    return lambda: emit_batch(f3t, wfts, pnums, pdent)
                pending_batch = make_batch()

                def make_finals(j=j, pnums=pnums, pdent=pdent):
                    def finals():
                        rden = wpool.tile([128, WQ], f32, tag="rden", bufs=1)
                        # den in [1, 25]: no recip edge cases; 51 ULP is fine
                        nc.vector.reciprocal_approx_fast(out=rden, in_=pdent)
                        osb = opool.tile([128, 3, WQ], f32, tag="osb")
                        for ch in range(3):
                            nc.vector.tensor_mul(out=osb[:, ch, :],
                                                 in0=pnums[ch], in1=rden)
                        nc.sync.dma_start(out=out[j], in_=osb)
                    return finals
                prev_finals = make_finals()
            pending_batch()
            prev_finals()

    nc.compile()
    return nc


def prep_inputs(input, coeffs, n_chunks=NCH):
    """Build per-core in_maps (list of 8 dicts of numpy arrays)."""
    inp = np.asarray(input, np.float32)
    f = inp[:, :C]                      # [2,32,720,1280]
    scale = inp[:, C:]                  # [2,34,720,1280]
    k = np.exp(np.asarray(coeffs, np.float32).reshape(-1))   # [34]
    sp = np.logaddexp(0.0, scale)
    params = -(k[None, :, None, None] * sp)
    pc = params[:, :C]
    psy = params[:, C]                  # [2,720,1280]
    psx = params[:, C + 1]

    # padded f: rows/cols shifted by +6
    fp = np.full((B, C, H + 12, W + 12), PADV, np.float32)
    fp[:, :, 6:6 + H, 6:6 + W] = f
    # padded first-3-channel f for the pixel stage: shifted by +6
    f3p = np.full((B, 3, H + 12, W + 12), PADV, np.float32)
    f3p[:, :, 6:6 + H, 6:6 + W] = f[:, :3]

    # selection master matrix: sel[(g,c), v] = 1 iff v == S0 + 4g
    sel = np.zeros((128, MW), np.float32)
    for g in range(NG):
        sel[32 * g:32 * (g + 1), S0 + 4 * g] = 1.0
    ident = np.eye(128, dtype=np.float32)

    # row-gather index with holes -> clamp to row 0 and zero later
    prow = np.where(REAL, PPERM, 0)

    in_maps = []
    for b in range(B):
        for q in range(4):
            x0 = WQ * q
            fpb = fp[b, :, :, x0:x0 + FW]          # [32, 732, 332]
            s = fpb.strides
            fin = np.lib.stride_tricks.as_strided(
                fpb, shape=(n_chunks, NG, C, FH, FW),
                strides=(CH * s[1], NY * s[1], s[0], s[1], s[2]),
            ).reshape(n_chunks, 128, FH, FW)

            pcb = pc[b, :, :, x0:x0 + WQ]          # [32, 720, 320]
            s = pcb.strides
            pcin = np.lib.stride_tricks.as_strided(
                pcb, shape=(n_chunks, NG, C, NY, WQ),
                strides=(CH * s[1], NY * s[1], s[0], s[1], s[2]),
            ).reshape(n_chunks, 128, NY, WQ)

            # f3in[j, d, p, c, xx] = f3p[b, c, 120j + prow[p] + 3(d-2) + 6, x0+xx]
            j_idx = np.arange(n_chunks)[:, None, None]
            d_idx = np.arange(5)[None, :, None]
            p_idx = prow[None, None, :]
            rows = CH * j_idx + p_idx + 3 * (d_idx - 2) + 6   # [j, d, p]
            f3in = f3p[b][:, rows, x0:x0 + PXW]               # [3, j, d, p, PXW]
            # -> [j, p, d, c, x] to match SBUF tile [128, 5, 3, PXW]
            f3in = np.ascontiguousarray(f3in.transpose(1, 3, 2, 0, 4))
            f3in[:, ~REAL] = 0.0

            # spin[j, p, m, xx]: spatial coefficient maps in pixel layout,
            # channels [psy, psx, 4*psy, 4*psx]
            rows2 = CH * np.arange(n_chunks)[:, None] + prow[None, :]  # [j, p]
            spy = psy[b][rows2, x0:x0 + WQ]                   # [j, p, WQ]
            spx = psx[b][rows2, x0:x0 + WQ]
            spin = np.stack([spy, spx, 4.0 * spy, 4.0 * spx], axis=2)
            spin[:, ~REAL] = 0.0

            in_maps.append({
                "fin": fin.astype(BF16),
                "pcin": pcin.astype(BF16),
                "f3in": f3in.astype(BF16),
                "spin": np.ascontiguousarray(spin).astype(BF16),
                "selin": sel.astype(BF16),
                "idin": ident.astype(BF16),
            })
    return in_maps


def assemble_output(results, n_chunks=NCH):
    outf = np.empty((B, 3, H, W), np.float32)
    i = 0
    for b in range(B):
        for q in range(4):
            x0 = WQ * q
            o = np.asarray(results[i]["out"], np.float32)  # [j, 128, 3, WQ]
            for j in range(n_chunks):
                # fancy-index on axis 2 with slice on axis 1 -> result axes
                # are (row, c, x), matching o[j, REAL] directly
                outf[b, :, CH * j + PPERM[REAL], x0:x0 + WQ] = o[j, REAL]
            i += 1
    return outf


_NC_CACHE = {}


def kernel(input, coeffs, kernel_size=5, dilation=3, dynamic_size=3):
    assert int(kernel_size) == 5 and int(dilation) == 3
    assert int(dynamic_size) == 3
    from concourse import bass_utils

    if "nc" not in _NC_CACHE:
        _NC_CACHE["nc"] = build_nc(NCH)
    nc = _NC_CACHE["nc"]
    in_maps = prep_inputs(input, coeffs, NCH)
    res = bass_utils.run_bass_kernel_spmd(nc, in_maps,
                                          core_ids=list(range(NCORE)))
    return assemble_output(res.results, NCH)
